# revision 41
# baseline (speedup 1.0000x reference)
"""Trainium2 Bass kernel for the SLAYER-style 2-layer spiking MLP.

Reference computation (per batch element n):
    flat   = input.reshape(64, 3072)
    a1     = flat @ w1.T                      (constant over time)
    u1[t]  = a1 * c[t]          where c = cumsum(srm kernel)  (PSP of a
             time-constant input is just a ramp scale)
    s1     = spike_scan(u1)     sequential threshold w/ refractory feedback
    a2[t]  = w2 @ s1[:, t]
    u2     = psp(a2)            (true temporal conv, srm kernel)
    out    = spike_scan(u2)

Key algebraic facts exploited on-device:
  * The refractory kernel rk[d] = -20*d*e^{1-d} (d=1..32) is
    polynomial-geometric, so the refractory sum r[t] = sum_d rk[d] s[t-d]
    follows an exact order-2 recurrence:
        P[t] = q*P[t-1] + s[t-1]
        R[t] = q*R[t-1] + P[t]          (q = e^-1, states scaled by -1/20)
        spike:  u + (-20)*R >= theta  <=>  R + 0.5 <= u/20
    The d>32 truncation of rk is ~1e-11 and far below fp32 noise.
  * Each scan step is exactly 3 fused scalar_tensor_tensor DVE ops over a
    [104, 33] tile holding both layers (layer 2 rides along lagged LAG
    steps).  The serial 3-op chain at ~229ns/op pitch is the hard floor
    (DVE pipeline drain); everything else is kept off the DVE: the u1/20
    table comes off the PE (outer product a1 x c20 via a c20 (x) I8
    moving operand, evacuated by ACT), the per-block a2 restack
    [10,(t,n)] -> [80,t] goes through a DRAM-bounce DMA pair (row
    convention o2*8+n makes both DMA sides canonical), and the psp2
    filter is two chained TTS scans (W1, then u2[t] = p*u2[t-1]+W1[t],
    the second writing the lagged layer-2 column of U directly).
  * The SRM PSP filter srm[k] = (k/10)e^{1-k/10} is the same confluent
    geometric form, handled by two hardware tensor_tensor_scan ops per
    8-step block (never truncated within T=100, so it is exact).
  * fc1 is fp32 on the PE (fp32r is ~tf32 precision and flips spikes);
    w1 streams in 7 pieces with per-piece tiles so the 24 accumulating
    matmuls pipeline behind the DMA, and 5 throwaway matmuls ramp the
    PE clock while the first piece is in flight (fp32 halves drop from
    ~889ns cold to ~593ns).

Sharding: data-parallel over batch, 8 elements per core, weights replicated.
"""

import numpy as np

NB = 8            # batch elements per core
T = 100           # timesteps
B = 8             # pipeline block size
LAG = 18          # layer-2 ride-along lag
TF = T + LAG      # fused scan steps
NCOL = 33         # 32 layer-1 columns (4 chunks x 8 batch) + 1 layer-2 column
PMAX = 104        # padded partition count per o-chunk
MC = [103, 103, 102, 102]      # o-chunk sizes (sum = 410)
OFF = [0, 103, 206, 308]
KT = 24           # 3072 / 128 k-tiles
NO1 = 410
NO2 = 10
TCH = 34          # u-table t-chunk size (chunks 34/34/32, free>=256 for fp32r)

_CACHE = {}


def _consts():
    q = float(np.float32(np.exp(-1.0)))          # refractory ratio
    p = float(np.float32(np.exp(-0.1)))          # SRM ratio
    k2 = float(np.float32(np.exp(1.0) / 200.0))  # a2 pre-scale: u2/20 = sum
    return q, p, k2


def build():
    import concourse.bass as bass
    import concourse.bacc as bacc
    import concourse.mybir as mybir
    from concourse import tile

    f32 = mybir.dt.float32
    f32r = mybir.dt.float32r
    Alu = mybir.AluOpType
    q, p, _ = _consts()

    nc = bacc.Bacc("TRN2", target_bir_lowering=False, debug=False, num_devices=8)

    flatL_d = nc.dram_tensor("flatL", [128, KT * NB], f32, kind="ExternalInput")
    w1L_d = nc.dram_tensor("w1L", [128, KT * NO1], f32, kind="ExternalInput")
    w2p_d = nc.dram_tensor("w2p", [PMAX, 4, NO2], f32, kind="ExternalInput")
    kron_d = nc.dram_tensor("kron", [NB, T, NB], f32, kind="ExternalInput")
    pc_d = nc.dram_tensor("pconst", [80, B], f32, kind="ExternalInput")
    out_d = nc.dram_tensor("out", [80, T], f32, kind="ExternalOutput")
    scr_d = nc.dram_tensor("a2scr", [NO2, NB, T], f32, kind="Internal")

    with tile.TileContext(nc) as tc:
        with (
            tc.tile_pool(name="pers", bufs=1) as pool,
            tc.tile_pool(name="ps1", bufs=1, space="PSUM") as ps1,
            tc.tile_pool(name="psu", bufs=2, space="PSUM") as psu,
            tc.tile_pool(name="ps2", bufs=2, space="PSUM") as ps2,
        ):
            PIECES = [1, 1, 2, 4, 4, 4, 8]
            POFF = [0, 1, 2, 4, 8, 12, 16]
            w1p = [
                pool.tile([128, PIECES[g], NO1], f32, tag=f"w1p{g}",
                          name=f"w1p{g}")
                for g in range(7)
            ]

            fTsb = pool.tile([128, KT, NB], f32, tag="fTsb")
            w2sb = pool.tile([PMAX, 4, NO2], f32, tag="w2sb")
            kronsb = pool.tile([NB, T, NB], f32, tag="kronsb")
            pcsb = pool.tile([80, B], f32, tag="pcsb")
            U = pool.tile([PMAX, TF, NCOL], f32, tag="U")
            # mega-tile: spike history S (TF+1 slots of NCOL) followed by the
            # interleaved IIR state [P(NCOL) | R(NCOL)] — one address space so
            # dual-range APs can address them together.
            SW = (TF + 1) * NCOL
            M = pool.tile([PMAX, SW + 2 * NCOL], f32, tag="M")
            a1rsb = pool.tile([NB, NO1], f32, tag="a1rsb")
            warm = pool.tile([128, 512], f32, tag="warm")
            a2tmp = pool.tile([NO2, NB, B], f32, tag="a2tmp")
            a2s = pool.tile([80, T + 1], f32, tag="a2s")
            ostage = pool.tile([80, T], f32, tag="ostage")
            W1 = pool.tile([80, T + 1], f32, tag="W1")

            # ---- state init (before any DMA lands) ----
            nc.gpsimd.memset(warm[:], 0.0)
            nc.gpsimd.memset(U[:], -1.0)
            nc.vector.memset(U[0:80, LAG - 1, 32:33], 0.0)
            nc.vector.memset(M[:, 0:NCOL], 0.0)            # S slot 0
            nc.vector.memset(M[:, SW:SW + 2 * NCOL], 0.0)  # P | R
            nc.vector.memset(a2s[:, 0:1], A2SHIFT)

            nc.vector.memset(W1[:, 0:1], 0.0)

            # ---- input DMAs: small tensors first (all contiguous), then w1
            # in 6 large pieces so fc1 matmuls pipeline behind the stream ----
            nc.scalar.dma_start(
                fTsb[:], flatL_d[:].rearrange("p (k n) -> p k n", n=NB)
            )
            w1v = w1L_d[:].rearrange("p (k o) -> p k o", o=NO1)
            for g in range(7):
                nc.scalar.dma_start(
                    w1p[g][:], w1v[:, POFF[g]:POFF[g] + PIECES[g], :]
                )
            nc.sync.dma_start(w2sb[:], w2p_d[:])
            nc.sync.dma_start(kronsb[:], kron_d[:])
            nc.sync.dma_start(pcsb[:], pc_d[:])

            # ---- fc1: a1row[n, o] = flat @ w1.T, accumulated over k ----
            kpiece = []
            for g in range(7):
                kpiece += [(g, i) for i in range(PIECES[g])]
            # ramp the PE clock while the first w1 piece is in flight
            wps = ps1.tile([128, 512], f32, tag="wps", name="wps")
            for _ in range(5):
                nc.tensor.matmul(
                    wps[:], warm[:, 0:128], warm[:], start=True, stop=True
                )
            a1row = ps1.tile([NB, NO1], f32, tag="a1row", name="a1row")
            for k in range(KT):
                g, i = kpiece[k]
                nc.tensor.matmul(
                    a1row[:],
                    fTsb[:, k, :],
                    w1p[g][:, i, :],
                    start=(k == 0),
                    stop=(k == KT - 1),
                )
            # ---- u1/20 table via PE outer product:
            # U[o1, t, n] (chunk c) = sum_n' a1rsb[n', OFF[c]+o1] * c20[t]δnn'
            # moving operand = c20 (x) I8, stationary = a1 rows. ----
            def emit_u_chunk(t0, t1):
                for c in range(4):
                    ups = psu.tile([PMAX, TCH, NB], f32, tag=f"ups{c % 2}", name="ups")
                    nc.tensor.matmul(
                        ups[0:MC[c], 0:t1 - t0, :],
                        a1rsb[0:NB, OFF[c]:OFF[c] + MC[c]],
                        kronsb[:, t0:t1, :],
                        start=True,
                        stop=True,
                    )
                    nc.scalar.activation(
                        U[0:MC[c], t0:t1, c * NB:(c + 1) * NB],
                        ups[0:MC[c], 0:t1 - t0, :],
                        mybir.ActivationFunctionType.Copy,
                    )

            # evac a1 (4 chunk evacs back-to-back), then the first U
            # matmuls, then their evacs -- queues overlap maximally
            for c in range(4):
                nc.scalar.activation(
                    a1rsb[0:NB, OFF[c]:OFF[c] + MC[c]],
                    a1row[0:NB, OFF[c]:OFF[c] + MC[c]],
                    mybir.ActivationFunctionType.Copy,
                )
            for c in range(4):
                ups = psu.tile([PMAX, TCH, NB], f32, tag=f"ups{c % 2}",
                               name="ups")
                nc.tensor.matmul(
                    ups[0:MC[c], 0:8, :],
                    a1rsb[0:NB, OFF[c]:OFF[c] + MC[c]],
                    kronsb[:, 0:8, :],
                    start=True,
                    stop=True,
                )
                nc.scalar.activation(
                    U[0:MC[c], 0:8, c * NB:(c + 1) * NB],
                    ups[0:MC[c], 0:8, :],
                    mybir.ActivationFunctionType.Copy,
                )
            emit_u_chunk(8, 16)

            # ---- fused scan: layer-1 at step tau, layer-2 at tau-LAG ----
            MW = M.ap[0][0]          # mega-tile row stride (elements)
            MOFF = M.offset

            def m_ap(off, dims, parts=PMAX):
                return bass.AP(M.tensor, MOFF + off, [[MW, parts]] + dims)

            r_in = m_ap(SW + NCOL, [[1, NCOL]])         # R
            p_st = m_ap(SW, [[1, NCOL]])                # P

            DELAY = 3
            DELAY2 = 6
            blocks = []
            for b in range((T + B - 1) // B):
                blocks.append((b * B, min((b + 1) * B, T)))
            block_at = {tb1 - 1 + DELAY: (tb0, tb1) for tb0, tb1 in blocks}
            scan_at = {
                tb1 - 1 + (DELAY2 + 2 if tb0 == 0 else DELAY2): (tb0, tb1)
                for tb0, tb1 in blocks
            }
            uchunk_at = {11: (16, 50), 18: (50, 84), 40: (84, 100)}

            for tau in range(TF):
                if tau < T:
                    # full width: 32 layer-1 columns + layer-2 column
                    pp, rr, w0, off = p_st, r_in, NCOL, 0
                else:
                    # tail: layer-1 finished, only column 32 is live
                    pp = m_ap(SW + 32, [[1, 1]])
                    rr = m_ap(SW + NCOL + 32, [[1, 1]])
                    w0, off = 1, 32
                # P = q*P + s_{tau-1}
                nc.vector.scalar_tensor_tensor(
                    pp, pp, q, m_ap(tau * NCOL + off, [[1, w0]]),
                    Alu.mult, Alu.add,
                )
                # R = q*R + P
                nc.vector.scalar_tensor_tensor(
                    rr, rr, q, pp, Alu.mult, Alu.add,
                )
                # s_{tau} = (R + 0.5) <= u/20
                nc.vector.scalar_tensor_tensor(
                    m_ap((tau + 1) * NCOL + off, [[1, w0]]),
                    rr,
                    0.5,
                    U[:, tau, off:off + w0],
                    Alu.add,
                    Alu.is_le,
                )

                if tau in uchunk_at:
                    emit_u_chunk(*uchunk_at[tau])

                if tau == LAG + 66:
                    nc.scalar.activation(
                        ostage[:, 0:64],
                        m_ap((LAG + 1) * NCOL + 32, [[NCOL, 64]], parts=80),
                        mybir.ActivationFunctionType.Copy,
                    )
                    nc.sync.dma_start(out_d[:, 0:64], ostage[:, 0:64])

                if tau in block_at:
                    tb0, tb1 = block_at[tau]
                    blk = tb1 - tb0
                    # a2[o2, t, n] for t in [tb0, tb1): 4 chunk-matmuls
                    a2ps = ps2.tile([NO2, B, NB], f32, tag="a2ps", name="a2ps")
                    for c in range(4):
                        nc.tensor.matmul(
                            a2ps[:, 0:blk, :],
                            w2sb[:, c, :],
                            m_ap((tb0 + 1) * NCOL + c * NB,
                                 [[NCOL, blk], [1, NB]]),
                            start=(c == 0),
                            stop=(c == 3),
                        )
                    # evac (w2 is pre-scaled by e/200 on host); write
                    # permuted [o2][n][t] so t is contiguous for the DMA
                    APW = a2tmp.ap[0][0]
                    nc.scalar.activation(
                        bass.AP(a2tmp.tensor, a2tmp.offset,
                                [[APW, NO2], [1, blk], [B, NB]]),
                        a2ps[:, 0:blk, :],
                        mybir.ActivationFunctionType.Copy,
                    )
                    # re-stack [10,(t,n)] -> [80, t] with row = o2*8+n via a
                    # DRAM bounce: scr is [o2][n][t], which IS row-major
                    # [80, T] under the o2-major row convention.
                    nc.sync.dma_start(
                        scr_d[:, :, tb0:tb1], a2tmp[:, :, 0:blk]
                    )
                    nc.sync.dma_start(
                        a2s[:, tb0 + 1:tb1 + 1],
                        scr_d[:].rearrange("o n t -> (o n) t")[:, tb0:tb1],
                    )

                if tau in scan_at:
                    tb0, tb1 = scan_at[tau]
                    blk = tb1 - tb0
                    # W1[t] = (a2s[t-1] + W1[t-1]) * p   (hardware scan)
                    nc.vector.tensor_tensor_scan(
                        W1[:, tb0 + 1:tb1 + 1],
                        a2s[:, tb0:tb1],
                        pcsb[:, 0:blk],
                        W1[:, tb0:tb0 + 1],
                        Alu.add,
                        Alu.mult,
                    )
                    # u2[t] = p*u2[t-1] + W1[t]  (W2[t] == p*u2[t-1]),
                    # written directly into the lagged layer-2 column of U
                    nc.vector.tensor_tensor_scan(
                        U[0:80, tb0 + LAG:tb1 + LAG, 32],
                        pcsb[:, 0:blk],
                        W1[:, tb0 + 1:tb1 + 1],
                        U[0:80, tb0 + LAG - 1, 32:33],
                        Alu.mult,
                        Alu.add,
                    )

            # ---- output: layer-2 spikes, fused steps LAG..LAG+T ----
            nc.scalar.activation(
                ostage[:, 64:T],
                m_ap((LAG + 65) * NCOL + 32, [[NCOL, T - 64]], parts=80),
                mybir.ActivationFunctionType.Copy,
            )
            nc.sync.dma_start(out_d[:, 64:T], ostage[:, 64:T])

    nc.compile()
    return nc


def _host_inputs(input, w1, w2):
    f32 = np.float32
    _, p, k2 = _consts()
    flat = input.reshape(64, -1).astype(f32)                  # (64, 3072)
    # flatL[p, k, n] = flat[n, k*128+p]  (per-core n-slices taken later)
    flatL = np.ascontiguousarray(
        flat.T.reshape(KT, 128, 64).transpose(1, 0, 2)        # (128, KT, 64)
    )
    # w1L[p, k, o] = w1[o, k*128+p]
    w1L = np.ascontiguousarray(
        w1.astype(f32).T.reshape(KT, 128, NO1).transpose(1, 0, 2)
        .reshape(128, KT * NO1)
    )
    w2p = np.zeros((PMAX, 4, NO2), f32)
    for c in range(4):
        w2p[0:MC[c], c, :] = (w2.astype(f32)[:, OFF[c]:OFF[c] + MC[c]].T) * k2
    t = np.arange(T, dtype=np.float64)
    srm = (t / 10.0) * np.exp(1.0 - t / 10.0)
    c20 = (np.cumsum(srm) / 20.0).astype(f32)
    kron = np.zeros((NB, T, NB), f32)
    for n in range(NB):
        kron[n, :, n] = c20
    pconst = np.full((80, B), p, f32)
    return flatL, w1L, w2p, kron, pconst


def kernel(input, w1, w2):
    from concourse.bass_utils import run_bass_kernel_spmd

    if "nc" not in _CACHE:
        _CACHE["nc"] = build()
    nc = _CACHE["nc"]

    flatL, w1L, w2p, kron, pconst = _host_inputs(input, w1, w2)
    in_maps = []
    for core in range(8):
        in_maps.append({
            "flatL": np.ascontiguousarray(
                flatL[:, :, core * NB:(core + 1) * NB]
            ).reshape(128, KT * NB),
            "w1L": w1L,
            "w2p": w2p,
            "kron": kron,
            "pconst": pconst,
        })
    res = run_bass_kernel_spmd(nc, in_maps, core_ids=list(range(8)))
    full = np.zeros((64, NO2, T), np.float32)
    for core in range(8):
        full[core * NB:(core + 1) * NB] = (
            res.results[core]["out"].reshape(NO2, NB, T).transpose(1, 0, 2)
        )
    return full


# revision 42
# speedup vs baseline: 1.0247x; 1.0247x over previous
"""Trainium2 Bass kernel for the SLAYER-style 2-layer spiking MLP.

Reference computation (per batch element n):
    flat   = input.reshape(64, 3072)
    a1     = flat @ w1.T                      (constant over time)
    u1[t]  = a1 * c[t]          where c = cumsum(srm kernel)  (PSP of a
             time-constant input is just a ramp scale)
    s1     = spike_scan(u1)     sequential threshold w/ refractory feedback
    a2[t]  = w2 @ s1[:, t]
    u2     = psp(a2)            (true temporal conv, srm kernel)
    out    = spike_scan(u2)

Key algebraic facts exploited on-device:
  * The refractory kernel rk[d] = -20*d*e^{1-d} (d=1..32) is
    polynomial-geometric, so the refractory sum r[t] = sum_d rk[d] s[t-d]
    follows an exact order-2 recurrence:
        P[t] = q*P[t-1] + s[t-1]
        R[t] = q*R[t-1] + P[t]          (q = e^-1, states scaled by -1/20)
        spike:  u + (-20)*R >= theta  <=>  R + 0.5 <= u/20
    The d>32 truncation of rk is ~1e-11 and far below fp32 noise.
  * Each scan step is exactly 3 fused scalar_tensor_tensor DVE ops over a
    [104, 33] tile holding both layers (layer 2 rides along lagged LAG
    steps).  The serial 3-op chain at ~229ns/op pitch is the hard floor
    (DVE pipeline drain); everything else is kept off the DVE: the u1/20
    table comes off the PE (outer product a1 x c20 via a c20 (x) I8
    moving operand, evacuated by ACT), the per-block a2 restack
    [10,(t,n)] -> [80,t] goes through a DRAM-bounce DMA pair (row
    convention o2*8+n makes both DMA sides canonical), and the psp2
    filter is two chained TTS scans (W1, then u2[t] = p*u2[t-1]+W1[t],
    the second writing the lagged layer-2 column of U directly).
  * The SRM PSP filter srm[k] = (k/10)e^{1-k/10} is the same confluent
    geometric form, handled by two hardware tensor_tensor_scan ops per
    8-step block (never truncated within T=100, so it is exact).
  * fc1 is fp32 on the PE (fp32r is ~tf32 precision and flips spikes);
    w1 streams in 7 pieces with per-piece tiles so the 24 accumulating
    matmuls pipeline behind the DMA, and 5 throwaway matmuls ramp the
    PE clock while the first piece is in flight (fp32 halves drop from
    ~889ns cold to ~593ns).

Sharding: data-parallel over batch, 8 elements per core, weights replicated.
"""

import numpy as np

NB = 8            # batch elements per core
T = 100           # timesteps
B = 8             # pipeline block size
LAG = 18          # layer-2 ride-along lag
TF = T + LAG      # fused scan steps
NCOL = 33         # 32 layer-1 columns (4 chunks x 8 batch) + 1 layer-2 column
PMAX = 104        # padded partition count per o-chunk
MC = [103, 103, 102, 102]      # o-chunk sizes (sum = 410)
OFF = [0, 103, 206, 308]
KT = 24           # 3072 / 128 k-tiles
NO1 = 410
NO2 = 10
TCH = 34          # u-table t-chunk size (chunks 34/34/32, free>=256 for fp32r)

_CACHE = {}


def _consts():
    q = float(np.float32(np.exp(-1.0)))          # refractory ratio
    p = float(np.float32(np.exp(-0.1)))          # SRM ratio
    k2 = float(np.float32(np.exp(1.0) / 200.0))  # a2 pre-scale: u2/20 = sum
    return q, p, k2


def build():
    import concourse.bass as bass
    import concourse.bacc as bacc
    import concourse.mybir as mybir
    from concourse import tile

    f32 = mybir.dt.float32
    f32r = mybir.dt.float32r
    Alu = mybir.AluOpType
    q, p, _ = _consts()

    nc = bacc.Bacc("TRN2", target_bir_lowering=False, debug=False, num_devices=8)

    flatL_d = nc.dram_tensor("flatL", [128, KT * NB], f32, kind="ExternalInput")
    w1L_d = nc.dram_tensor("w1L", [128, KT * NO1], f32, kind="ExternalInput")
    w2p_d = nc.dram_tensor("w2p", [PMAX, 4, NO2], f32, kind="ExternalInput")
    kron_d = nc.dram_tensor("kron", [NB, T, NB], f32, kind="ExternalInput")
    pc_d = nc.dram_tensor("pconst", [80, B], f32, kind="ExternalInput")
    out_d = nc.dram_tensor("out", [80, T], f32, kind="ExternalOutput")
    scr_d = nc.dram_tensor("a2scr", [NO2, NB, T], f32, kind="Internal")

    with tile.TileContext(nc) as tc:
        with (
            tc.tile_pool(name="pers", bufs=1) as pool,
            tc.tile_pool(name="ps1", bufs=1, space="PSUM") as ps1,
            tc.tile_pool(name="psu", bufs=2, space="PSUM") as psu,
            tc.tile_pool(name="ps2", bufs=2, space="PSUM") as ps2,
        ):
            PIECES = [1, 1, 2, 4, 4, 4, 8]
            POFF = [0, 1, 2, 4, 8, 12, 16]
            w1p = [
                pool.tile([128, PIECES[g], NO1], f32, tag=f"w1p{g}",
                          name=f"w1p{g}")
                for g in range(7)
            ]

            fTsb = pool.tile([128, KT, NB], f32, tag="fTsb")
            w2sb = pool.tile([PMAX, 4, NO2], f32, tag="w2sb")
            kronsb = pool.tile([NB, T, NB], f32, tag="kronsb")
            pcsb = pool.tile([80, B], f32, tag="pcsb")
            U = pool.tile([PMAX, TF, NCOL], f32, tag="U")
            # mega-tile: spike history S (TF+1 slots of NCOL) followed by the
            # interleaved IIR state [P(NCOL) | R(NCOL)] — one address space so
            # dual-range APs can address them together.
            SW = (TF + 1) * NCOL
            M = pool.tile([PMAX, SW + 2 * NCOL], f32, tag="M")
            a1rsb = pool.tile([NB, NO1], f32, tag="a1rsb")
            warm = pool.tile([128, 512], f32, tag="warm")
            a2tmp = pool.tile([NO2, NB, B], f32, tag="a2tmp")
            a2s = pool.tile([80, T + 1], f32, tag="a2s")
            ostage = pool.tile([80, T], f32, tag="ostage")
            W1 = pool.tile([80, T + 1], f32, tag="W1")

            # ---- state init (before any DMA lands) ----
            nc.gpsimd.memset(warm[:], 0.0)
            nc.gpsimd.memset(U[:], -1.0)
            nc.vector.memset(U[0:80, LAG - 1, 32:33], 0.0)
            nc.vector.memset(M[:, 0:NCOL], 0.0)            # S slot 0
            nc.vector.memset(M[:, SW:SW + 2 * NCOL], 0.0)  # P | R
            nc.vector.memset(a2s[:, 0:1], A2SHIFT)

            nc.vector.memset(W1[:, 0:1], 0.0)

            # ---- input DMAs: small tensors first (all contiguous), then w1
            # in 6 large pieces so fc1 matmuls pipeline behind the stream ----
            nc.scalar.dma_start(
                fTsb[:], flatL_d[:].rearrange("p (k n) -> p k n", n=NB)
            )
            w1v = w1L_d[:].rearrange("p (k o) -> p k o", o=NO1)
            for g in range(7):
                nc.scalar.dma_start(
                    w1p[g][:], w1v[:, POFF[g]:POFF[g] + PIECES[g], :]
                )
            nc.sync.dma_start(w2sb[:], w2p_d[:])
            nc.sync.dma_start(kronsb[:], kron_d[:])
            nc.sync.dma_start(pcsb[:], pc_d[:])

            # ---- fc1: a1row[n, o] = flat @ w1.T, accumulated over k ----
            kpiece = []
            for g in range(7):
                kpiece += [(g, i) for i in range(PIECES[g])]
            # ramp the PE clock while the first w1 piece is in flight
            wps = ps1.tile([128, 512], f32, tag="wps", name="wps")
            for _ in range(5):
                nc.tensor.matmul(
                    wps[:], warm[:, 0:128], warm[:], start=True, stop=True
                )
            a1row = ps1.tile([NB, NO1], f32, tag="a1row", name="a1row")
            for k in range(KT):
                g, i = kpiece[k]
                nc.tensor.matmul(
                    a1row[:],
                    fTsb[:, k, :],
                    w1p[g][:, i, :],
                    start=(k == 0),
                    stop=(k == KT - 1),
                )
            # ---- u1/20 table via PE outer product:
            # U[o1, t, n] (chunk c) = sum_n' a1rsb[n', OFF[c]+o1] * c20[t]δnn'
            # moving operand = c20 (x) I8, stationary = a1 rows. ----
            def emit_u_chunk(t0, t1):
                for c in range(4):
                    ups = psu.tile([PMAX, TCH, NB], f32, tag=f"ups{c % 2}", name="ups")
                    nc.tensor.matmul(
                        ups[0:MC[c], 0:t1 - t0, :],
                        a1rsb[0:NB, OFF[c]:OFF[c] + MC[c]],
                        kronsb[:, t0:t1, :],
                        start=True,
                        stop=True,
                    )
                    nc.scalar.activation(
                        U[0:MC[c], t0:t1, c * NB:(c + 1) * NB],
                        ups[0:MC[c], 0:t1 - t0, :],
                        mybir.ActivationFunctionType.Copy,
                    )

            # evac a1 (4 chunk evacs back-to-back), then the first U
            # matmuls, then their evacs -- queues overlap maximally
            for c in range(4):
                nc.scalar.activation(
                    a1rsb[0:NB, OFF[c]:OFF[c] + MC[c]],
                    a1row[0:NB, OFF[c]:OFF[c] + MC[c]],
                    mybir.ActivationFunctionType.Copy,
                )
            for c in range(4):
                ups = psu.tile([PMAX, TCH, NB], f32, tag=f"ups{c % 2}",
                               name="ups")
                nc.tensor.matmul(
                    ups[0:MC[c], 0:8, :],
                    a1rsb[0:NB, OFF[c]:OFF[c] + MC[c]],
                    kronsb[:, 0:8, :],
                    start=True,
                    stop=True,
                )
                nc.scalar.activation(
                    U[0:MC[c], 0:8, c * NB:(c + 1) * NB],
                    ups[0:MC[c], 0:8, :],
                    mybir.ActivationFunctionType.Copy,
                )
            emit_u_chunk(8, 16)

            # ---- fused scan: layer-1 at step tau, layer-2 at tau-LAG ----
            MW = M.ap[0][0]          # mega-tile row stride (elements)
            MOFF = M.offset

            def m_ap(off, dims, parts=PMAX):
                return bass.AP(M.tensor, MOFF + off, [[MW, parts]] + dims)

            r_in = m_ap(SW + NCOL, [[1, NCOL]])         # R
            p_st = m_ap(SW, [[1, NCOL]])                # P

            DELAY = 3
            DELAY2 = 6
            blocks = []
            for b in range((T + B - 1) // B):
                blocks.append((b * B, min((b + 1) * B, T)))
            block_at = {tb1 - 1 + DELAY: (tb0, tb1) for tb0, tb1 in blocks}
            scan_at = {
                tb1 - 1 + (DELAY2 + 2 if tb0 == 0 else DELAY2): (tb0, tb1)
                for tb0, tb1 in blocks
            }
            uchunk_at = {11: (16, 50), 18: (50, 84), 40: (84, 100)}

            for tau in range(TF):
                if tau < T:
                    # full width: 32 layer-1 columns + layer-2 column
                    pp, rr, w0, off = p_st, r_in, NCOL, 0
                else:
                    # tail: layer-1 finished, only column 32 is live
                    pp = m_ap(SW + 32, [[1, 1]])
                    rr = m_ap(SW + NCOL + 32, [[1, 1]])
                    w0, off = 1, 32
                # P = q*P + s_{tau-1}
                nc.vector.scalar_tensor_tensor(
                    pp, pp, q, m_ap(tau * NCOL + off, [[1, w0]]),
                    Alu.mult, Alu.add,
                )
                # R = q*R + P
                nc.vector.scalar_tensor_tensor(
                    rr, rr, q, pp, Alu.mult, Alu.add,
                )
                # s_{tau} = (R + 0.5) <= u/20
                nc.vector.scalar_tensor_tensor(
                    m_ap((tau + 1) * NCOL + off, [[1, w0]]),
                    rr,
                    0.5,
                    U[:, tau, off:off + w0],
                    Alu.add,
                    Alu.is_le,
                )

                if tau in uchunk_at:
                    emit_u_chunk(*uchunk_at[tau])

                if tau == LAG + 66:
                    nc.scalar.activation(
                        ostage[:, 0:64],
                        m_ap((LAG + 1) * NCOL + 32, [[NCOL, 64]], parts=80),
                        mybir.ActivationFunctionType.Copy,
                    )
                    nc.sync.dma_start(out_d[:, 0:64], ostage[:, 0:64])
                if tau == LAG + 94:
                    nc.scalar.activation(
                        ostage[:, 64:92],
                        m_ap((LAG + 65) * NCOL + 32, [[NCOL, 28]], parts=80),
                        mybir.ActivationFunctionType.Copy,
                    )
                    nc.sync.dma_start(out_d[:, 64:92], ostage[:, 64:92])

                if tau in block_at:
                    tb0, tb1 = block_at[tau]
                    blk = tb1 - tb0
                    # a2[o2, t, n] for t in [tb0, tb1): 4 chunk-matmuls
                    a2ps = ps2.tile([NO2, B, NB], f32, tag="a2ps", name="a2ps")
                    for c in range(4):
                        nc.tensor.matmul(
                            a2ps[:, 0:blk, :],
                            w2sb[:, c, :],
                            m_ap((tb0 + 1) * NCOL + c * NB,
                                 [[NCOL, blk], [1, NB]]),
                            start=(c == 0),
                            stop=(c == 3),
                        )
                    # evac (w2 is pre-scaled by e/200 on host); write
                    # permuted [o2][n][t] so t is contiguous for the DMA
                    APW = a2tmp.ap[0][0]
                    nc.scalar.activation(
                        bass.AP(a2tmp.tensor, a2tmp.offset,
                                [[APW, NO2], [1, blk], [B, NB]]),
                        a2ps[:, 0:blk, :],
                        mybir.ActivationFunctionType.Copy,
                    )
                    # re-stack [10,(t,n)] -> [80, t] with row = o2*8+n via a
                    # DRAM bounce: scr is [o2][n][t], which IS row-major
                    # [80, T] under the o2-major row convention.
                    nc.sync.dma_start(
                        scr_d[:, :, tb0:tb1], a2tmp[:, :, 0:blk]
                    )
                    nc.sync.dma_start(
                        a2s[:, tb0 + 1:tb1 + 1],
                        scr_d[:].rearrange("o n t -> (o n) t")[:, tb0:tb1],
                    )

                if tau in scan_at:
                    tb0, tb1 = scan_at[tau]
                    blk = tb1 - tb0
                    # W1[t] = (a2s[t-1] + W1[t-1]) * p   (hardware scan)
                    nc.vector.tensor_tensor_scan(
                        W1[:, tb0 + 1:tb1 + 1],
                        a2s[:, tb0:tb1],
                        pcsb[:, 0:blk],
                        W1[:, tb0:tb0 + 1],
                        Alu.add,
                        Alu.mult,
                    )
                    # u2[t] = p*u2[t-1] + W1[t]  (W2[t] == p*u2[t-1]),
                    # written directly into the lagged layer-2 column of U
                    nc.vector.tensor_tensor_scan(
                        U[0:80, tb0 + LAG:tb1 + LAG, 32],
                        pcsb[:, 0:blk],
                        W1[:, tb0 + 1:tb1 + 1],
                        U[0:80, tb0 + LAG - 1, 32:33],
                        Alu.mult,
                        Alu.add,
                    )

            # ---- output: layer-2 spikes, fused steps LAG..LAG+T ----
            nc.scalar.activation(
                ostage[:, 92:T],
                m_ap((LAG + 93) * NCOL + 32, [[NCOL, T - 92]], parts=80),
                mybir.ActivationFunctionType.Copy,
            )
            nc.sync.dma_start(out_d[:, 92:T], ostage[:, 92:T])

    nc.compile()
    return nc


def _host_inputs(input, w1, w2):
    f32 = np.float32
    _, p, k2 = _consts()
    flat = input.reshape(64, -1).astype(f32)                  # (64, 3072)
    # flatL[p, k, n] = flat[n, k*128+p]  (per-core n-slices taken later)
    flatL = np.ascontiguousarray(
        flat.T.reshape(KT, 128, 64).transpose(1, 0, 2)        # (128, KT, 64)
    )
    # w1L[p, k, o] = w1[o, k*128+p]
    w1L = np.ascontiguousarray(
        w1.astype(f32).T.reshape(KT, 128, NO1).transpose(1, 0, 2)
        .reshape(128, KT * NO1)
    )
    w2p = np.zeros((PMAX, 4, NO2), f32)
    for c in range(4):
        w2p[0:MC[c], c, :] = (w2.astype(f32)[:, OFF[c]:OFF[c] + MC[c]].T) * k2
    t = np.arange(T, dtype=np.float64)
    srm = (t / 10.0) * np.exp(1.0 - t / 10.0)
    c20 = (np.cumsum(srm) / 20.0).astype(f32)
    kron = np.zeros((NB, T, NB), f32)
    for n in range(NB):
        kron[n, :, n] = c20
    pconst = np.full((80, B), p, f32)
    return flatL, w1L, w2p, kron, pconst


def kernel(input, w1, w2):
    from concourse.bass_utils import run_bass_kernel_spmd

    if "nc" not in _CACHE:
        _CACHE["nc"] = build()
    nc = _CACHE["nc"]

    flatL, w1L, w2p, kron, pconst = _host_inputs(input, w1, w2)
    in_maps = []
    for core in range(8):
        in_maps.append({
            "flatL": np.ascontiguousarray(
                flatL[:, :, core * NB:(core + 1) * NB]
            ).reshape(128, KT * NB),
            "w1L": w1L,
            "w2p": w2p,
            "kron": kron,
            "pconst": pconst,
        })
    res = run_bass_kernel_spmd(nc, in_maps, core_ids=list(range(8)))
    full = np.zeros((64, NO2, T), np.float32)
    for core in range(8):
        full[core * NB:(core + 1) * NB] = (
            res.results[core]["out"].reshape(NO2, NB, T).transpose(1, 0, 2)
        )
    return full
